# revision 1
# baseline (speedup 1.0000x reference)
"""DigitCaps routing-by-agreement kernel for 8 Trainium2 NeuronCores.

Math (faithful to the reference):
  u_hat[b,j,n,d] = sum_e x[b,n,e] W[j,n,d,e]
  iter1: c1 = 0.1 exactly (softmax of zeros)
         s1 = 0.1 * sum_n u_hat          -> GEMM, no u_hat materialization
         v1 = squash(s1)                 (GLOBAL scalar norm -> host reduce)
  iter2: t1[b,j,n] = sum_d v1 u_hat      -> per-j GEMM (A = v1 @ W) + DVE mul/reduce
         c2 = softmax_j(t1)
         s2 = sum_n c2 u_hat             -> y = c2*x then per-j GEMM vs W
         v  = squash(s2)                 (global scalar -> host epilogue)

Sharding: pure data-parallel over batch (64 samples/core), W replicated.
Two NEFF launches; the tiny global squash scalar between iterations is
reduced on the host (s1 is only [512,160] f32).

All big operands are bf16 on chip with fp32 PSUM accumulation; softmax
logits t1 kept fp32.
"""

import numpy as np
import ml_dtypes
from contextlib import ExitStack

import concourse.bass as bass
import concourse.bacc as bacc
import concourse.tile as tile
import concourse.mybir as mybir
from concourse.bass_utils import run_bass_kernel_spmd

MCORES = 8
B, N, E, J, D = 512, 1152, 8, 10, 16
BC = B // MCORES            # 64 samples per core
NE = N * E                  # 9216
JD = J * D                  # 160
KC = NE // 128              # 72 k-chunks for the s1 GEMM
NCH = N // 128              # 9 n-chunks
EPS = 1e-7

F32 = mybir.dt.float32
BF16 = mybir.dt.bfloat16
AX = mybir.AxisListType
ALU = mybir.AluOpType
ACTF = mybir.ActivationFunctionType

_BF = ml_dtypes.bfloat16

# column extents inside the concatenated launch-B inputs
_WA_COLS = J * NCH * 128            # 11520
_WS_COLS = J * E * NCH * D          # 11520
_XE_COLS = NCH * E * BC             # 4608
_VZ_COLS = E * J * BC               # 5120


def _bass():
    # Bacc (not raw Bass): its compile() runs generate_event_semaphores,
    # which splits multi-wait sync lists into the 1-wait-per-instruction
    # form this walrus requires.
    return bacc.Bacc("TRN2", target_bir_lowering=False, debug=False,
                     num_devices=MCORES)


def build_launch_a():
    """s1_raw[b, (j d)] = sum_{(n e)} xT[(n e), b] * Wk[(n e), (j d)].

    Inputs (host layout, bf16):
      xT2  [128, KC*BC]  : xT[(n e), b] chunked -> col (k*BC + b), k-chunk k
      Wk2  [128, KC*JD]  : Wk[(n e), (j d)] chunked likewise
    Output: s1_raw [BC, JD] f32.
    """
    nc = _bass()
    xT2 = nc.dram_tensor("xT2", [128, KC * BC], BF16, kind="ExternalInput").ap()
    Wk2 = nc.dram_tensor("Wk2", [128, KC * JD], BF16, kind="ExternalInput").ap()
    s1 = nc.dram_tensor("s1", [BC, JD], F32, kind="ExternalOutput").ap()

    with tile.TileContext(nc) as tc, ExitStack() as ctx:
        io = ctx.enter_context(tc.tile_pool(name="io", bufs=1))
        ps = ctx.enter_context(tc.tile_pool(name="ps", bufs=1, space="PSUM"))
        sb = ctx.enter_context(tc.tile_pool(name="sb", bufs=1))

        xT_sb = io.tile([128, KC * BC], BF16)
        Wk_sb = io.tile([128, KC * JD], BF16)
        for i in range(6):
            lo, hi = i * KC // 6, (i + 1) * KC // 6
            nc.sync.dma_start(xT_sb[:, lo * BC:hi * BC],
                              xT2[:, lo * BC:hi * BC])
            nc.scalar.dma_start(Wk_sb[:, lo * JD:hi * JD],
                                Wk2[:, lo * JD:hi * JD])

        acc = ps.tile([BC, JD], F32)
        for k in range(KC):
            nc.tensor.matmul(
                acc[:],
                lhsT=xT_sb[:, k * BC:(k + 1) * BC],
                rhs=Wk_sb[:, k * JD:(k + 1) * JD],
                start=(k == 0), stop=(k == KC - 1),
            )
        out_sb = sb.tile([BC, JD], F32)
        nc.scalar.copy(out_sb[:], acc[:])
        nc.sync.dma_start(s1, out_sb[:])
    nc.compile()
    return nc


def build_launch_b():
    """Routing iteration 2, fully on chip except the squash scalars.

    Inputs (host layout, bf16, concatenated column-wise):
      WB [128, 23040] = [ WA | WS ]
        WA block (j,ch): [128,128] lhsT, rows e*16+d = W[j, ch*128+n', d, e]
        WS slice (j,e,ch): [128, D] lhsT, row n' = W[j, ch*128+n', d, e]
      xv [128, 9728] = [ v1z | xE_h0 | xE_h1 ]
        v1z col j*512 + e*64 + b; rows e*16..+16 = v1T[d,b] for j (j-major
        so the j=0 slice can be DMA'd first)
        xE_h[n=ch*128+p, (ch,e,b')] = x[h*32+b', n, e], per-half contiguous
    Output: s2_raw [D, J*BC] f32  (s2_raw[d, j*BC+b] = s2[b, j, d])
    """
    nc = _bass()
    WB = nc.dram_tensor("WB", [128, _WA_COLS + _WS_COLS], BF16,
                        kind="ExternalInput").ap()
    xv = nc.dram_tensor("xv", [128, _XE_COLS + _VZ_COLS], BF16,
                        kind="ExternalInput").ap()
    s2 = nc.dram_tensor("s2", [D, J * BC], F32, kind="ExternalOutput").ap()

    EB = E * BC  # 512

    HB = BC // 2          # 32: sub-batch half, pipelines B1->B2->B3
    EH = E * HB           # 256 columns per (ch, half) block
    CW = NCH * EH         # 2304: full (ch,e,b') width per half

    with tile.TileContext(nc) as tc, ExitStack() as ctx:
        io = ctx.enter_context(tc.tile_pool(name="io", bufs=1))
        psA = ctx.enter_context(tc.tile_pool(name="psA", bufs=3, space="PSUM"))
        psS = ctx.enter_context(tc.tile_pool(name="psS", bufs=2, space="PSUM"))
        stage = ctx.enter_context(tc.tile_pool(name="stage", bufs=4))
        soft = ctx.enter_context(tc.tile_pool(name="soft", bufs=3))
        big = ctx.enter_context(tc.tile_pool(name="big", bufs=1))

        WB_sb = io.tile([128, _WA_COLS + _WS_COLS], BF16)
        xv_sb = io.tile([128, _XE_COLS + _VZ_COLS], BF16)
        # DMA pieces ordered by first use: v1z j=0, WA quarter 1, xE half
        # 0, the rest of v1z, remaining WA, xE half 1, WS (B3-only) last.
        VJ = E * BC                       # one j-block of v1z
        q = _WA_COLS // 4
        CWc = NCH * E * (BC // 2)         # xE half width
        def dmas():
            # sync queue: WAq1, v1z-rest, WAq2..4; scalar queue: v1z-j0,
            # xE halves, WS.  The sim round-robins the queues, so this
            # interleaving lands each piece just before its first use.
            yield nc.sync, WB_sb, WB, 0, q
            yield nc.scalar, xv_sb, xv, 0, 2 * VJ
            yield nc.sync, xv_sb, xv, _VZ_COLS, _VZ_COLS + CWc
            yield nc.scalar, xv_sb, xv, 2 * VJ, _VZ_COLS
            yield nc.sync, WB_sb, WB, q, 2 * q
            yield nc.scalar, xv_sb, xv, _VZ_COLS + CWc, _VZ_COLS + 2 * CWc
            yield nc.sync, WB_sb, WB, 2 * q, 3 * q
            yield nc.scalar, WB_sb, WB, _WA_COLS, _WA_COLS + _WS_COLS // 2
            yield nc.sync, WB_sb, WB, 3 * q, 4 * q
            yield nc.scalar, WB_sb, WB, _WA_COLS + _WS_COLS // 2, \
                _WA_COLS + _WS_COLS
        for eng, dst, srcT, lo, hi in dmas():
            eng.dma_start(dst[:, lo:hi], srcT[:, lo:hi])

        def WA_blk(j, ch):
            o = (j * NCH + ch) * 128
            return WB_sb[:, o:o + 128]

        def WS_slc(j, e, ch):
            o = _WA_COLS + ((j * E + e) * NCH + ch) * D
            return WB_sb[:, o:o + D]

        # v1z region viewed [p, j, e, b] (j-major layout)
        v1_v = xv_sb[:, 0:_VZ_COLS] \
            .rearrange("p (jj e b) -> p jj e b", e=E, jj=J)

        # per-half logits/coefficients, col = ch*(J*HB) + j*HB + b'
        t1_h = [big.tile([128, NCH * J * HB], BF16, tag=f"t1{h}",
                          name=f"t1_h{h}") for h in range(2)]
        c2_h = [big.tile([128, NCH * J * HB], BF16, tag=f"c2{h}",
                         name=f"c2_h{h}") for h in range(2)]
        s2_sb = soft.tile([D, J * BC], F32, tag="s2")

        def xE_h(h):
            o = _VZ_COLS + h * CW
            return xv_sb[:, o:o + CW].rearrange(
                "p (c e b) -> p c e b", c=NCH, e=E)            # [p,9,8,32]

        def b1_unit(h, j):
            rhs = v1_v[:, j][:, :, h * HB:(h + 1) * HB]        # [p,8,32]
            Ps = stage.tile([128, CW], BF16, tag="Ps", name=f"Ps_{h}_{j}")
            for ch2 in range((NCH + 2) // 3):
                lo, hi = ch2 * 3, min(NCH, ch2 * 3 + 3)
                acc = psA.tile([128, (hi - lo) * EH], F32, tag="pA",
                               name=f"acc_{h}_{j}_{ch2}")
                for ch in range(lo, hi):
                    nc.tensor.matmul(
                        acc[:, (ch - lo) * EH:(ch - lo + 1) * EH]
                            .rearrange("p (e b) -> p e b", e=E),
                        lhsT=WA_blk(j, ch), rhs=rhs,
                        start=True, stop=True,
                    )
                nc.scalar.copy(Ps[:, lo * EH:hi * EH], acc[:])
            # P = Ps * x, then binary tree over e within each EH block
            Pm = stage.tile([128, CW], BF16, tag="Pm", name=f"Pm_{h}_{j}")
            nc.vector.tensor_mul(
                Pm[:].rearrange("p (c e b) -> p c e b", c=NCH, e=E),
                Ps[:].rearrange("p (c e b) -> p c e b", c=NCH, e=E),
                xE_h(h),
            )
            T1 = stage.tile([128, NCH * 4 * HB], BF16, tag="T1",
                            name=f"T1_{h}_{j}")
            nc.vector.tensor_add(
                T1[:].rearrange("p (c k) -> p c k", k=4 * HB),
                Pm[:].rearrange("p (c k) -> p c k", k=EH)[:, :, 0:4 * HB],
                Pm[:].rearrange("p (c k) -> p c k", k=EH)[:, :, 4 * HB:EH],
            )
            T2 = stage.tile([128, NCH * 2 * HB], BF16, tag="T2",
                            name=f"T2_{h}_{j}")
            nc.vector.tensor_add(
                T2[:].rearrange("p (c k) -> p c k", k=2 * HB),
                T1[:].rearrange("p (c k) -> p c k", k=4 * HB)[:, :, 0:2 * HB],
                T1[:].rearrange("p (c k) -> p c k", k=4 * HB)[:, :, 2 * HB:4 * HB],
            )
            nc.vector.tensor_add(
                t1_h[h][:].rearrange("p (c k) -> p c k", k=J * HB)
                    [:, :, j * HB:(j + 1) * HB],
                T2[:].rearrange("p (c k) -> p c k", k=2 * HB)[:, :, 0:HB],
                T2[:].rearrange("p (c k) -> p c k", k=2 * HB)[:, :, HB:2 * HB],
            )

        def b2_unit(h):
            # whole-half softmax: exp, sum over j, reciprocal, normalize
            Ex = soft.tile([128, NCH * J * HB], BF16, tag="Ex",
                           name=f"Ex_{h}")
            Se = soft.tile([128, NCH * HB], F32, tag="Se", name=f"Se_{h}")
            # two chunk-ranges so the j-sum starts at the exp halfway point
            for lo, hi in ((0, 5), (5, NCH)):
                nc.scalar.activation(
                    Ex[:, lo * J * HB:hi * J * HB],
                    t1_h[h][:, lo * J * HB:hi * J * HB], ACTF.Exp)
                nc.vector.tensor_reduce(
                    Se[:, lo * HB:hi * HB]
                        .rearrange("p (c b) -> p c b", c=hi - lo),
                    Ex[:, lo * J * HB:hi * J * HB]
                        .rearrange("p (c j b) -> p c b j", c=hi - lo, j=J),
                    axis=AX.X, op=ALU.add,
                )
            Re = soft.tile([128, NCH * HB], F32, tag="Re", name=f"Re_{h}")
            nc.vector.reciprocal(Re[:], Se[:])
            Rb = soft.tile([128, NCH * HB], BF16, tag="Rb", name=f"Rb_{h}")
            nc.vector.tensor_copy(Rb[:], Re[:])
            nc.vector.tensor_mul(
                c2_h[h][:].rearrange("p (c j b) -> p c j b", c=NCH, j=J),
                Ex[:].rearrange("p (c j b) -> p c j b", c=NCH, j=J),
                Rb[:].rearrange("p (c b) -> p c b", c=NCH)
                    .unsqueeze(2).broadcast_to([128, NCH, J, HB]),
            )

        def b3_unit(h, j):
            y_j = stage.tile([128, CW], BF16, tag="yj", name=f"yj_{h}_{j}")
            nc.vector.tensor_mul(
                y_j[:].rearrange("p (c e b) -> p c e b", c=NCH, e=E),
                xE_h(h),
                c2_h[h][:].rearrange("p (c k) -> p c k", k=J * HB)
                    [:, :, j * HB:(j + 1) * HB]
                    .unsqueeze(2).broadcast_to([128, NCH, E, HB]),
            )
            acc2 = psS.tile([D, HB], F32, tag="pS", name=f"acc2_{h}_{j}")
            for ch in range(NCH):
                for e in range(E):
                    nc.tensor.matmul(
                        acc2[:],
                        lhsT=WS_slc(j, e, ch),
                        rhs=y_j[:, (ch * E + e) * HB:(ch * E + e + 1) * HB],
                        start=(ch == 0 and e == 0),
                        stop=(ch == NCH - 1 and e == E - 1),
                    )
            nc.scalar.copy(
                s2_sb[:, j * BC + h * HB:j * BC + (h + 1) * HB], acc2[:])

        # half 0 logits+softmax; then interleave its B3 with half 1's B1 so
        # the PE/ACT work of one hides under the DVE work of the other.
        for j in range(J):
            b1_unit(0, j)
        b2_unit(0)
        for j in range(J):
            b3_unit(0, j)
            b1_unit(1, j)
        b2_unit(1)
        for j in range(J):
            b3_unit(1, j)
        nc.sync.dma_start(s2, s2_sb[:])
    nc.compile()
    return nc


_cache = {}


def _get_programs():
    if "a" not in _cache:
        _cache["a"] = build_launch_a()
        _cache["b"] = build_launch_b()
    return _cache["a"], _cache["b"]


def _prep_host(x, W):
    xf = np.ascontiguousarray(x, dtype=np.float32)
    Wf = np.ascontiguousarray(W, dtype=np.float32)

    # Launch A weights: Wk[(n e), (j d)] = W[j,n,d,e], chunked to [128, KC*JD]
    Wk = Wf.transpose(1, 3, 0, 2).reshape(NE, JD)
    Wk2 = np.ascontiguousarray(
        Wk.reshape(KC, 128, JD).transpose(1, 0, 2).reshape(128, KC * JD)
    ).astype(_BF)

    # WA block (j,ch): rows e*16+d, cols n' -> W[j, ch*128+n', d, e]
    WAt = Wf.transpose(3, 2, 0, 1).reshape(E * D, J, NCH, 128)  # [(e d), j, ch, n']
    WA = np.ascontiguousarray(WAt.reshape(E * D, J * NCH * 128))

    # WS slice (j,e,ch): [128, D] rows n' -> W[j, ch*128+n', d, e]
    WSt = Wf.transpose(1, 0, 3, 2).reshape(NCH, 128, J, E, D)   # [ch, n', j, e, d]
    WS = np.ascontiguousarray(
        WSt.transpose(1, 2, 3, 0, 4).reshape(128, J * E * NCH * D))

    WB = np.concatenate([WA, WS], axis=1).astype(_BF)           # [128, 23040]

    # Per-core x layouts
    xs = xf.reshape(MCORES, BC, N, E)
    xT2s, xEs = [], []
    for c in range(MCORES):
        xT = xs[c].transpose(1, 2, 0).reshape(NE, BC)           # [(n e), b]
        xT2s.append(np.ascontiguousarray(
            xT.reshape(KC, 128, BC).transpose(1, 0, 2).reshape(128, KC * BC)
        ).astype(_BF))
        xE = xs[c].transpose(1, 2, 0).reshape(N, E * BC)        # [n, (e b)]
        xEs.append(np.ascontiguousarray(
            xE.reshape(NCH, 128, E * BC).transpose(1, 0, 2)
              .reshape(128, NCH * E * BC)))
    return Wk2, WB, xT2s, xEs


def kernel(x, W):
    nc_a, nc_b = _get_programs()
    Wk2, WB, xT2s, xEs = _prep_host(x, W)
    core_ids = list(range(MCORES))

    in_a = [{"xT2": xT2s[c], "Wk2": Wk2} for c in core_ids]
    res_a = run_bass_kernel_spmd(nc_a, in_a, core_ids).results
    s1_raw = np.stack([res_a[c]["s1"] for c in core_ids])       # [M, BC, JD]

    s1 = 0.1 * s1_raw.reshape(B, J, D).astype(np.float32)
    sq1 = float(np.sum(s1.astype(np.float64) ** 2))
    g1 = sq1 / (1.0 + sq1) / np.sqrt(sq1 + EPS)
    v1 = (g1 * s1).astype(np.float32)                           # [B, J, D]

    # v1z per core (j-major): col j*512+e*64+b; rows e*16+d = v1T[d,b]
    v1T = v1.reshape(MCORES, BC, J, D)
    in_b = []
    for c in range(MCORES):
        v1z = np.zeros((128, J * E * BC), np.float32)
        vt = v1T[c].transpose(2, 1, 0)                          # [d, j, b]
        v4 = v1z.reshape(E, D, J, E, BC)
        for e in range(E):
            v4[e, :, :, e, :] = vt.transpose(0, 1, 2)[:, :, :] \
                .reshape(D, J, BC).transpose(0, 1, 2)[:, :, :] if False else vt
        v1z = v4.reshape(128, J * E * BC)
        xE9 = xEs[c].reshape(128, NCH, E, BC)
        xh0 = np.ascontiguousarray(xE9[:, :, :, 0:BC // 2]).reshape(128, -1)
        xh1 = np.ascontiguousarray(xE9[:, :, :, BC // 2:]).reshape(128, -1)
        xvc = np.concatenate([v1z, xh0, xh1], axis=1).astype(_BF)
        in_b.append({"WB": WB, "xv": xvc})
    res_b = run_bass_kernel_spmd(nc_b, in_b, core_ids).results
    s2_raw = np.stack([res_b[c]["s2"] for c in core_ids])       # [M, D, J*BC]

    s2 = s2_raw.reshape(MCORES, D, J, BC).transpose(0, 3, 2, 1).reshape(B, J, D)
    s2 = s2.astype(np.float32)
    sq2 = float(np.sum(s2.astype(np.float64) ** 2))
    g2 = sq2 / (1.0 + sq2) / np.sqrt(sq2 + EPS)
    return (g2 * s2).astype(np.float32)

